# revision 2
# baseline (speedup 1.0000x reference)
"""Trainium2 Bass kernel for nn_NodeProcessor (GNN message passing).

Computes, for full inputs:
    agg = segment_sum(edges, receivers, N)          # [N, Fe]
    gb  = globals_[node_graph_ids]                  # [N, Fg]
    out = relu(concat([nodes, agg, gb]) @ W + b)    # [N, Fout]

Strategy (8 NeuronCores, SPMD, no collectives):
  - Host sorts edges by receiver and shards NODES by contiguous receiver
    range (N/8 nodes per core); each core gets exactly the edges that
    target its nodes, so the segment-sum is fully local.
  - Per core, nodes are bucketed into classes k = ceil(deg/4); each node's
    edge list is padded to 4k slots. The padded edge stream is laid out so
    the device computes 4:1 slot sums with a single stationary one-hot
    matrix on the TensorEngine, accumulating k passes per node into PSUM:
        psum[32 nodes, 48*tiles] += ones4.T @ edge_chunk[128, 48*tiles]
  - Edges are streamed as bf16 hi + bf16 lo (exact-ish fp32 split: two
    accumulating matmuls), preserving ~1e-5 accuracy at 2x bf16 rate.
  - The per-node aggregate [node, 48] is PE-transposed to [48, node] and
    consumed by the fused output layer:
        out_t[128, nodes] = relu(Wn.T @ nodes_t + We.T @ agg_t + G''.T @ onehot)
    where G'' = globals_ @ Wg + b is computed on device ([8, Fout]).
  - Host inverse-permutes the per-core transposed outputs into [N, Fout].
"""

import numpy as np
import ml_dtypes

import concourse.bass as bass
import concourse.tile as tile
from concourse import bacc, mybir
from concourse import bass_utils

F32 = mybir.dt.float32
F32R = mybir.dt.float32r
BF16 = mybir.dt.bfloat16

N_CORES = 8
BLOCK = 4          # edge slots summed per one-hot matmul output
TPB = 10           # tiles (32-node groups) per psum bank fill -> N = 480
GROUP_TILES = 32   # max tiles per edge DMA
FINAL_CHUNK = 512  # nodes per output matmul chunk
FE = 48
FOUT = 128
FNODE = 128

# 'f32' = exact fp32 final matmul (4 cyc/row); 'f32r' = tf32-like (1 cyc/row,
# ~2e-4 rel err contribution)
FINAL_MODE = "f32"


def _split_hilo(x_f32):
    hi = x_f32.astype(ml_dtypes.bfloat16)
    lo = (x_f32 - hi.astype(np.float32)).astype(ml_dtypes.bfloat16)
    return hi, lo


def host_prep(nodes, edges, globals_, W, b, receivers, node_graph_ids):
    nodes = np.ascontiguousarray(np.asarray(nodes, dtype=np.float32))
    edges = np.ascontiguousarray(np.asarray(edges, dtype=np.float32))
    globals_ = np.asarray(globals_, dtype=np.float32)
    W = np.asarray(W, dtype=np.float32)
    b = np.asarray(b, dtype=np.float32)
    recv = np.asarray(receivers).astype(np.int64)
    gid = np.asarray(node_graph_ids).astype(np.int64)

    N, FN = nodes.shape
    E, fe = edges.shape
    NG, FG = globals_.shape
    assert FN == FNODE and fe == FE and N % N_CORES == 0
    NPC = N // N_CORES

    order = np.argsort(recv, kind="stable")
    deg = np.bincount(recv, minlength=N).astype(np.int64)
    starts = np.zeros(N + 1, np.int64)
    np.cumsum(deg, out=starts[1:])

    kcls = np.maximum((deg + BLOCK - 1) // BLOCK, 1).astype(np.int64)
    KMAX = int(kcls.max())

    counts = np.zeros((N_CORES, KMAX + 1), np.int64)
    for c in range(N_CORES):
        counts[c] = np.bincount(kcls[c * NPC:(c + 1) * NPC], minlength=KMAX + 1)
    bud = counts.max(axis=0)
    bud = ((bud + 31) // 32) * 32
    bud[0] = 0
    SN0 = int(bud.sum())
    bud[1] += (-SN0) % FINAL_CHUNK
    SN = int(bud.sum())
    G = SN // 32
    n_chunks = SN // FINAL_CHUNK

    k_of_sn = np.repeat(np.arange(KMAX + 1), bud)
    kg = k_of_sn[::32]  # class per 32-node group (uniform within group)

    n_bf = (G + TPB - 1) // TPB

    # chunk table: per bank-fill, the suffix-run matmul passes
    chunk_list = []  # (bank, j, toff, nbj, tile_col)
    col = 0
    bank_tiles = []
    for bb in range(n_bf):
        g0 = bb * TPB
        nt = min(TPB, G - g0)
        bank_tiles.append(nt)
        kmax_b = int(kg[g0 + nt - 1])
        for j in range(kmax_b):
            nbj = int((kg[g0:g0 + nt] > j).sum())
            chunk_list.append((bb, j, nt - nbj, nbj, col))
            col += nbj
    T_ALL = col

    # DMA groups: pack whole chunks up to GROUP_TILES tiles
    dma_groups = []  # (tile_start, tile_len)
    chunk_group = []  # per chunk: (group_idx, local_tile_off)
    cur_start, cur_len = 0, 0
    for (bb, j, toff, nbj, tcol) in chunk_list:
        if cur_len + nbj > GROUP_TILES:
            dma_groups.append((cur_start, cur_len))
            cur_start, cur_len = tcol, 0
        chunk_group.append((len(dma_groups), cur_len))
        cur_len += nbj
    if cur_len:
        dma_groups.append((cur_start, cur_len))

    # per-(entry tile) group id and pass j
    ent_g = np.empty(T_ALL, np.int64)
    ent_j = np.empty(T_ALL, np.int64)
    for (bb, j, toff, nbj, tcol) in chunk_list:
        g0 = bb * TPB
        ent_g[tcol:tcol + nbj] = g0 + toff + np.arange(nbj)
        ent_j[tcol:tcol + nbj] = j

    rows = np.arange(128)
    sn_mat = 32 * ent_g[:, None] + rows[None, :] // BLOCK   # [T_ALL, 128]
    sl_mat = BLOCK * ent_j[:, None] + rows[None, :] % BLOCK

    per_core = []
    for c in range(N_CORES):
        lo_n, hi_n = c * NPC, (c + 1) * NPC
        # class-sorted node stream
        sn_stream = np.full(SN, -1, np.int64)
        off = 0
        kc = kcls[lo_n:hi_n]
        ids = np.arange(lo_n, hi_n)
        for k in range(1, KMAX + 1):
            sel = ids[kc == k]
            sn_stream[off:off + len(sel)] = sel
            off += int(bud[k])
        # edge slab for this core (receiver-sorted rows) + zero row
        e0, e1 = int(starts[lo_n]), int(starts[hi_n])
        edges_local = np.concatenate(
            [edges[order[e0:e1]], np.zeros((1, FE), np.float32)], axis=0)
        ZR = e1 - e0

        n0 = sn_stream[sn_mat]                       # [T_ALL, 128]
        n0c = np.maximum(n0, 0)
        valid = (n0 >= 0) & (sl_mat < deg[n0c])
        ti = np.where(valid, starts[n0c] + sl_mat - e0, ZR).astype(np.int64)
        stream = edges_local[ti]                     # [T_ALL, 128, 48]
        stream = np.ascontiguousarray(stream.transpose(1, 0, 2)).reshape(128, T_ALL * FE)
        ehi, elo = _split_hilo(stream)

        # transposed node features / graph one-hot by stream position
        sel = sn_stream >= 0
        nt = np.zeros((SN, FNODE), np.float32)
        nt[sel] = nodes[sn_stream[sel]]
        nodes_t = np.ascontiguousarray(nt.T)
        oh = np.zeros((SN, NG), np.float32)
        oh[sel, gid[sn_stream[sel]]] = 1.0
        onehot_t = np.ascontiguousarray(oh.T)

        per_core.append(dict(
            edges_hi=ehi, edges_lo=elo, nodes_t=nodes_t, onehot_t=onehot_t,
            sn_stream=sn_stream,
        ))

    ones4 = np.zeros((128, 32), np.float32)
    ones4[np.arange(128), np.arange(128) // BLOCK] = 1.0
    consts = dict(
        ones4b=ones4.astype(ml_dtypes.bfloat16),
        ident32=np.eye(32, dtype=np.float32),
        W_n=np.ascontiguousarray(W[:FNODE]),
        W_e=np.ascontiguousarray(W[FNODE:FNODE + FE]),
        W_aug=np.ascontiguousarray(
            np.concatenate([W[FNODE + FE:], b[None, :]], axis=0)),  # [FG+1, FOUT]
        gat=np.ascontiguousarray(
            np.concatenate([globals_.T, np.ones((1, NG), np.float32)], axis=0)),  # [FG+1, NG]
    )
    tables = dict(
        N=N, E=E, NG=NG, FG=FG, SN=SN, G=G, T_ALL=T_ALL, n_bf=n_bf,
        n_chunks=n_chunks, KMAX=KMAX,
        bank_tiles=bank_tiles, chunk_list=chunk_list,
        dma_groups=dma_groups, chunk_group=chunk_group, kg=kg,
    )
    return tables, consts, per_core


def build_program(tables, consts):
    t = tables
    NG, FG = t["NG"], t["FG"]
    SN, G, T_ALL, n_bf = t["SN"], t["G"], t["T_ALL"], t["n_bf"]
    FDT = F32 if FINAL_MODE == "f32" else F32R

    nc = bacc.Bacc("TRN2", target_bir_lowering=False, debug=False,
                   num_devices=N_CORES)

    ehi_d = nc.dram_tensor("edges_hi", [128, T_ALL * FE], BF16, kind="ExternalInput").ap()
    elo_d = nc.dram_tensor("edges_lo", [128, T_ALL * FE], BF16, kind="ExternalInput").ap()
    nodes_d = nc.dram_tensor("nodes_t", [FNODE, SN], FDT, kind="ExternalInput").ap()
    oh_d = nc.dram_tensor("onehot_t", [NG, SN], FDT, kind="ExternalInput").ap()
    ones4_d = nc.dram_tensor("ones4b", [128, 32], BF16, kind="ExternalInput").ap()
    ident_d = nc.dram_tensor("ident32", [32, 32], F32, kind="ExternalInput").ap()
    wn_d = nc.dram_tensor("W_n", [FNODE, FOUT], FDT, kind="ExternalInput").ap()
    we_d = nc.dram_tensor("W_e", [FE, FOUT], FDT, kind="ExternalInput").ap()
    waug_d = nc.dram_tensor("W_aug", [FG + 1, FOUT], F32, kind="ExternalInput").ap()
    gat_d = nc.dram_tensor("gat", [FG + 1, NG], F32, kind="ExternalInput").ap()
    out_d = nc.dram_tensor("out_t", [FOUT, SN], F32, kind="ExternalOutput").ap()

    with tile.TileContext(nc) as tc:
        with (
            tc.tile_pool(name="consts", bufs=1) as cpool,
            tc.tile_pool(name="edges", bufs=4) as epool,
            tc.tile_pool(name="aggsb", bufs=n_bf) as apool,
            tc.tile_pool(name="comb", bufs=3) as combpool,
            tc.tile_pool(name="nodes", bufs=3) as npool,
            tc.tile_pool(name="oh", bufs=3) as ohpool,
            tc.tile_pool(name="outsb", bufs=3) as opool,
            tc.tile_pool(name="ps_agg", bufs=2, space=bass.MemorySpace.PSUM) as ps_agg,
            tc.tile_pool(name="ps_t", bufs=2, space=bass.MemorySpace.PSUM) as ps_t,
            tc.tile_pool(name="ps_o", bufs=2, space=bass.MemorySpace.PSUM) as ps_o,
            tc.tile_pool(name="ps_g", bufs=1, space=bass.MemorySpace.PSUM) as ps_g,
        ):
            ones4b = cpool.tile([128, 32], BF16)
            ident32 = cpool.tile([32, 32], F32)
            wn = cpool.tile([FNODE, FOUT], FDT)
            we = cpool.tile([FE, FOUT], FDT)
            waug = cpool.tile([FG + 1, FOUT], F32)
            gat = cpool.tile([FG + 1, NG], F32)
            for sb_t, d in [(ones4b, ones4_d), (ident32, ident_d), (wn, wn_d),
                            (we, we_d), (waug, waug_d), (gat, gat_d)]:
                nc.sync.dma_start(sb_t[:], d[:])

            # G'' = globals_ @ Wg + b  -> [NG, FOUT]
            pg = ps_g.tile([NG, FOUT], F32)
            nc.tensor.matmul(pg[:], gat[:], waug[:], start=True, stop=True)
            g2 = cpool.tile([NG, FOUT], FDT)
            nc.vector.tensor_copy(g2[:], pg[:])

            # ---- level 1: segment sums into agg_sb tiles [32, 48*nt] ----
            group_tiles = {}  # group idx -> (hi_tile, lo_tile, tile_start)
            agg_tiles = []
            ci = 0
            for bb in range(n_bf):
                nt = t["bank_tiles"][bb]
                p = ps_agg.tile([32, FE * TPB], F32)
                passes = [cl for cl in t["chunk_list"] if cl[0] == bb]
                for idx, (bb_, j, toff, nbj, tcol) in enumerate(passes):
                    gidx, loc = t["chunk_group"][ci]
                    ci += 1
                    if gidx not in group_tiles:
                        ts, tl = t["dma_groups"][gidx]
                        hi_t = epool.tile([128, FE * GROUP_TILES], BF16, tag="ehi")
                        lo_t = epool.tile([128, FE * GROUP_TILES], BF16, tag="elo")
                        nc.sync.dma_start(hi_t[:, :FE * tl], ehi_d[:, FE * ts:FE * (ts + tl)])
                        nc.sync.dma_start(lo_t[:, :FE * tl], elo_d[:, FE * ts:FE * (ts + tl)])
                        group_tiles[gidx] = (hi_t, lo_t)
                    hi_t, lo_t = group_tiles[gidx]
                    dst = p[:, FE * toff:FE * (toff + nbj)]
                    src_hi = hi_t[:, FE * loc:FE * (loc + nbj)]
                    src_lo = lo_t[:, FE * loc:FE * (loc + nbj)]
                    last = idx == len(passes) - 1
                    nc.tensor.matmul(dst, ones4b[:], src_hi,
                                     start=(j == 0), stop=False)
                    nc.tensor.matmul(dst, ones4b[:], src_lo,
                                     start=False, stop=last)
                asb = apool.tile([32, FE * TPB], F32)
                nc.vector.tensor_copy(asb[:, :FE * nt], p[:, :FE * nt])
                agg_tiles.append(asb)

            # ---- per 512-node chunk: transpose agg, final matmul, relu ----
            for c in range(t["n_chunks"]):
                pt = ps_t.tile([FE, FINAL_CHUNK], F32)
                for gg in range(FINAL_CHUNK // 32):
                    g = c * (FINAL_CHUNK // 32) + gg
                    bb, tt = g // TPB, g % TPB
                    nc.tensor.transpose(
                        pt[:, 32 * gg:32 * (gg + 1)],
                        agg_tiles[bb][:, FE * tt:FE * (tt + 1)],
                        ident32[:],
                    )
                comb = combpool.tile([FE, FINAL_CHUNK], FDT)
                nc.vector.tensor_copy(comb[:], pt[:])

                nodes_c = npool.tile([FNODE, FINAL_CHUNK], FDT)
                nc.sync.dma_start(nodes_c[:], nodes_d[:, FINAL_CHUNK * c:FINAL_CHUNK * (c + 1)])
                oh_c = ohpool.tile([NG, FINAL_CHUNK], FDT)
                nc.sync.dma_start(oh_c[:], oh_d[:, FINAL_CHUNK * c:FINAL_CHUNK * (c + 1)])

                po = ps_o.tile([FOUT, FINAL_CHUNK], F32)
                nc.tensor.matmul(po[:], wn[:], nodes_c[:], start=True, stop=False)
                nc.tensor.matmul(po[:], we[:], comb[:], start=False, stop=False)
                nc.tensor.matmul(po[:], g2[:], oh_c[:], start=False, stop=True)

                outc = opool.tile([FOUT, FINAL_CHUNK], F32)
                nc.scalar.activation(outc[:], po[:],
                                     mybir.ActivationFunctionType.Relu)
                nc.sync.dma_start(out_d[:, FINAL_CHUNK * c:FINAL_CHUNK * (c + 1)], outc[:])

    nc.compile()
    return nc


def _in_maps(tables, consts, per_core):
    maps = []
    fdt = np.float32
    for pc in per_core:
        maps.append({
            "edges_hi": pc["edges_hi"], "edges_lo": pc["edges_lo"],
            "nodes_t": pc["nodes_t"].astype(fdt), "onehot_t": pc["onehot_t"].astype(fdt),
            "ones4b": consts["ones4b"], "ident32": consts["ident32"],
            "W_n": consts["W_n"], "W_e": consts["W_e"],
            "W_aug": consts["W_aug"], "gat": consts["gat"],
        })
    return maps


def assemble_output(tables, per_core, results):
    N = tables["N"]
    out = np.zeros((N, FOUT), np.float32)
    for pc, r in zip(per_core, results):
        out_t = r["out_t"]
        sn = pc["sn_stream"]
        sel = sn >= 0
        out[sn[sel]] = out_t[:, sel].T
    return out


_CACHE = {}
TRACE = False          # set True (e.g. from test.py) to capture NTFF profile
LAST_RESULT = None     # BassKernelResults of the last kernel() call


def kernel(**inputs):
    global LAST_RESULT
    tables, consts, per_core = host_prep(
        inputs["nodes"], inputs["edges"], inputs["globals_"], inputs["W"],
        inputs["b"], inputs["receivers"], inputs["node_graph_ids"])
    key = (tables["SN"], tables["T_ALL"], tables["n_bf"], FINAL_MODE)
    if key not in _CACHE:
        _CACHE[key] = build_program(tables, consts)
    nc = _CACHE[key]
    kw = {}
    if TRACE:
        kw = dict(trace=True, trace_cores=list(range(N_CORES)))
    res = bass_utils.run_bass_kernel_spmd(
        nc, _in_maps(tables, consts, per_core), core_ids=list(range(N_CORES)),
        **kw)
    LAST_RESULT = res
    return assemble_output(tables, per_core, res.results)


def emulate(tables, consts, per_core):
    """Numpy emulation of the device program, from the prepped arrays."""
    t = tables
    outs = []
    g2 = consts["gat"].T @ consts["W_aug"]  # [NG, FOUT]
    for pc in per_core:
        stream = (pc["edges_hi"].astype(np.float32)
                  + pc["edges_lo"].astype(np.float32)).reshape(128, t["T_ALL"], FE)
        agg = np.zeros((t["SN"], FE), np.float32)
        for (bb, j, toff, nbj, tcol) in t["chunk_list"]:
            g0 = bb * TPB
            data = stream[:, tcol:tcol + nbj, :]            # [128, nbj, 48]
            sums = data.reshape(32, BLOCK, nbj, FE).sum(1)  # [32, nbj, 48]
            for tt in range(nbj):
                gsel = g0 + toff + tt
                agg[32 * gsel:32 * (gsel + 1)] += sums[:, tt, :]
        x = (pc["nodes_t"].T @ consts["W_n"]
             + agg @ consts["W_e"]
             + pc["onehot_t"].T @ g2)
        outs.append({"out_t": np.maximum(x, 0).T})
    return assemble_output(t, per_core, outs)


# revision 9
# speedup vs baseline: 1.5774x; 1.5774x over previous
"""Trainium2 Bass kernel for nn_NodeProcessor (GNN message passing).

Computes, for full inputs:
    agg = segment_sum(edges, receivers, N)          # [N, Fe]
    gb  = globals_[node_graph_ids]                  # [N, Fg]
    out = relu(concat([nodes, agg, gb]) @ W + b)    # [N, Fout]

Strategy (8 NeuronCores, SPMD, no collectives):
  - Host sorts edges by receiver and shards NODES by contiguous receiver
    range (N/8 nodes per core); each core gets exactly the edges that
    target its nodes, so the segment-sum is fully local.
  - Per core, nodes are bucketed into classes k = ceil(deg/4); each node's
    edge list is padded to 4k slots. The padded edge stream is laid out so
    the device computes 4:1 slot sums with a single stationary one-hot
    matrix on the TensorEngine, accumulating k passes per node into PSUM
    (psum layout [32 nodes, 8 groups x 64], 48 real cols per group):
        psum[32, g, 0:48] += ones4.T @ edge_chunk[128, 48*tiles]
  - Edges stream as bf16 hi + bf16 lo (near-exact fp32 split; two
    accumulating matmuls) on two DMA queues (Sync + Activation).
  - agg transpose to [feat, node] uses the DVE 32x32 block transpose +
    32-aligned cross-partition copies (no TensorEngine transposes).
  - Fused output layer per 512-node chunk:
        out_t[128, nodes] = relu(Wn.T @ nodes_t + We64.T @ comb + G''.T @ onehot)
    where G'' = globals_ @ Wg + b is computed on device ([8, Fout]) and
    comb[64, 512] holds agg feats (rows 48-63 zero-padded, We64 likewise).
  - Host inverse-permutes the per-core transposed outputs into [N, Fout].
"""

import numpy as np
import ml_dtypes

import concourse.bass as bass
import concourse.tile as tile
from concourse import bacc, mybir
from concourse import bass_utils

F32 = mybir.dt.float32
BF16 = mybir.dt.bfloat16

N_CORES = 8
BLOCK = 4          # edge slots summed per one-hot matmul output row
TPB = 8            # 32-node groups per psum bank fill (64-col strided blocks)
GROUP_TILES = 64   # tiles per edge DMA
FINAL_CHUNK = 512  # nodes per output matmul chunk
OUT_BATCH = 4      # chunks per output store DMA
FE = 48
FEP = 64           # padded agg feature pitch in psum/sbuf
FOUT = 128
FNODE = 128


def _split_hilo(x_f32):
    hi = x_f32.astype(ml_dtypes.bfloat16)
    lo = (x_f32 - hi.astype(np.float32)).astype(ml_dtypes.bfloat16)
    return hi, lo


def host_prep(nodes, edges, globals_, W, b, receivers, node_graph_ids):
    nodes = np.ascontiguousarray(np.asarray(nodes, dtype=np.float32))
    edges = np.ascontiguousarray(np.asarray(edges, dtype=np.float32))
    globals_ = np.asarray(globals_, dtype=np.float32)
    W = np.asarray(W, dtype=np.float32)
    b = np.asarray(b, dtype=np.float32)
    recv = np.asarray(receivers).astype(np.int64)
    gid = np.asarray(node_graph_ids).astype(np.int64)

    N, FN = nodes.shape
    E, fe = edges.shape
    NG, FG = globals_.shape
    assert FN == FNODE and fe == FE and N % N_CORES == 0
    NPC = N // N_CORES

    order = np.argsort(recv, kind="stable")
    deg = np.bincount(recv, minlength=N).astype(np.int64)
    starts = np.zeros(N + 1, np.int64)
    np.cumsum(deg, out=starts[1:])

    kcls = np.maximum((deg + BLOCK - 1) // BLOCK, 1).astype(np.int64)
    KMAX = int(kcls.max())

    counts = np.zeros((N_CORES, KMAX + 1), np.int64)
    for c in range(N_CORES):
        counts[c] = np.bincount(kcls[c * NPC:(c + 1) * NPC], minlength=KMAX + 1)
    bud = counts.max(axis=0)
    bud = ((bud + 31) // 32) * 32
    bud[0] = 0
    SN0 = int(bud.sum())
    bud[1] += (-SN0) % FINAL_CHUNK
    SN = int(bud.sum())
    G = SN // 32
    n_chunks = SN // FINAL_CHUNK

    k_of_sn = np.repeat(np.arange(KMAX + 1), bud)
    kg = k_of_sn[::32]  # class per 32-node group (uniform within group)

    n_bf = (G + TPB - 1) // TPB
    assert G % (FINAL_CHUNK // 32) == 0 and G % TPB == 0

    # chunk table: per bank-fill, the suffix-run matmul passes
    chunk_list = []  # (bank, j, toff, nbj, tile_col)
    col = 0
    bank_tiles = []
    for bb in range(n_bf):
        g0 = bb * TPB
        nt = min(TPB, G - g0)
        bank_tiles.append(nt)
        kmax_b = int(kg[g0 + nt - 1])
        for j in range(kmax_b):
            nbj = int((kg[g0:g0 + nt] > j).sum())
            chunk_list.append((bb, j, nt - nbj, nbj, col))
            col += nbj
    T_ALL = col

    # DMA groups: pack whole chunks up to GROUP_TILES tiles
    dma_groups = []  # (tile_start, tile_len)
    chunk_group = []  # per chunk: (group_idx, local_tile_off)
    cur_start, cur_len = 0, 0
    for (bb, j, toff, nbj, tcol) in chunk_list:
        if cur_len + nbj > GROUP_TILES:
            dma_groups.append((cur_start, cur_len))
            cur_start, cur_len = tcol, 0
        chunk_group.append((len(dma_groups), cur_len))
        cur_len += nbj
    if cur_len:
        dma_groups.append((cur_start, cur_len))

    ent_g = np.empty(T_ALL, np.int64)
    ent_j = np.empty(T_ALL, np.int64)
    for (bb, j, toff, nbj, tcol) in chunk_list:
        g0 = bb * TPB
        ent_g[tcol:tcol + nbj] = g0 + toff + np.arange(nbj)
        ent_j[tcol:tcol + nbj] = j

    rows = np.arange(128)
    sn_mat = 32 * ent_g[:, None] + rows[None, :] // BLOCK   # [T_ALL, 128]
    sl_mat = BLOCK * ent_j[:, None] + rows[None, :] % BLOCK

    per_core = []
    for c in range(N_CORES):
        lo_n, hi_n = c * NPC, (c + 1) * NPC
        sn_stream = np.full(SN, -1, np.int64)
        off = 0
        kc = kcls[lo_n:hi_n]
        ids = np.arange(lo_n, hi_n)
        for k in range(1, KMAX + 1):
            sel = ids[kc == k]
            sn_stream[off:off + len(sel)] = sel
            off += int(bud[k])
        e0, e1 = int(starts[lo_n]), int(starts[hi_n])
        edges_local = np.concatenate(
            [edges[order[e0:e1]], np.zeros((1, FE), np.float32)], axis=0)
        ZR = e1 - e0

        n0 = sn_stream[sn_mat]
        n0c = np.maximum(n0, 0)
        valid = (n0 >= 0) & (sl_mat < deg[n0c])
        ti = np.where(valid, starts[n0c] + sl_mat - e0, ZR).astype(np.int64)
        stream = edges_local[ti]                     # [T_ALL, 128, 48]
        stream = np.ascontiguousarray(stream.transpose(1, 0, 2)).reshape(128, T_ALL * FE)
        ehi, elo = _split_hilo(stream)

        sel = sn_stream >= 0
        nt = np.zeros((SN, FNODE), np.float32)
        nt[sel] = nodes[sn_stream[sel]]
        nodes_t = np.ascontiguousarray(nt.T)
        oh = np.zeros((SN, NG), np.float32)
        oh[sel, gid[sn_stream[sel]]] = 1.0
        onehot_t = np.ascontiguousarray(oh.T)

        per_core.append(dict(
            edges_hi=ehi, edges_lo=elo, nodes_t=nodes_t, onehot_t=onehot_t,
            sn_stream=sn_stream,
        ))

    ones4 = np.zeros((128, 32), np.float32)
    ones4[np.arange(128), np.arange(128) // BLOCK] = 1.0
    W_e64 = np.zeros((FEP, FOUT), np.float32)
    W_e64[:FE] = W[FNODE:FNODE + FE]
    consts = dict(
        ones4b=ones4.astype(ml_dtypes.bfloat16),
        W_n=np.ascontiguousarray(W[:FNODE]),
        W_e64=W_e64,
        W_aug=np.ascontiguousarray(
            np.concatenate([W[FNODE + FE:], b[None, :]], axis=0)),  # [FG+1, FOUT]
        gat=np.ascontiguousarray(
            np.concatenate([globals_.T, np.ones((1, NG), np.float32)], axis=0)),
        zeros128=np.zeros((32, 128), np.float32),
    )
    tables = dict(
        N=N, E=E, NG=NG, FG=FG, SN=SN, G=G, T_ALL=T_ALL, n_bf=n_bf,
        n_chunks=n_chunks, KMAX=KMAX,
        bank_tiles=bank_tiles, chunk_list=chunk_list,
        dma_groups=dma_groups, chunk_group=chunk_group, kg=kg,
    )
    return tables, consts, per_core


def build_program(tables, consts):
    t = tables
    NG, FG = t["NG"], t["FG"]
    SN, G, T_ALL, n_bf = t["SN"], t["G"], t["T_ALL"], t["n_bf"]
    n_chunks = t["n_chunks"]
    GPC = FINAL_CHUNK // 32          # groups per chunk (16)
    NB = FINAL_CHUNK * OUT_BATCH     # nodes per I/O batch

    nc = bacc.Bacc("TRN2", target_bir_lowering=False, debug=False,
                   num_devices=N_CORES)

    ehi_d = nc.dram_tensor("edges_hi", [128, T_ALL * FE], BF16, kind="ExternalInput").ap()
    elo_d = nc.dram_tensor("edges_lo", [128, T_ALL * FE], BF16, kind="ExternalInput").ap()
    nodes_d = nc.dram_tensor("nodes_t", [FNODE, SN], F32, kind="ExternalInput").ap()
    oh_d = nc.dram_tensor("onehot_t", [NG, SN], F32, kind="ExternalInput").ap()
    ones4_d = nc.dram_tensor("ones4b", [128, 32], BF16, kind="ExternalInput").ap()
    wn_d = nc.dram_tensor("W_n", [FNODE, FOUT], F32, kind="ExternalInput").ap()
    we_d = nc.dram_tensor("W_e64", [FEP, FOUT], F32, kind="ExternalInput").ap()
    waug_d = nc.dram_tensor("W_aug", [FG + 1, FOUT], F32, kind="ExternalInput").ap()
    gat_d = nc.dram_tensor("gat", [FG + 1, NG], F32, kind="ExternalInput").ap()
    z_d = nc.dram_tensor("zeros128", [32, 128], F32, kind="ExternalInput").ap()
    out_d = nc.dram_tensor("out_t", [FOUT, SN], F32, kind="ExternalOutput").ap()

    with tile.TileContext(nc) as tc:
        with (
            tc.tile_pool(name="consts", bufs=1) as cpool,
            tc.tile_pool(name="edges", bufs=3) as epool,
            tc.tile_pool(name="aggsb", bufs=6) as apool,
            tc.tile_pool(name="shsb", bufs=6) as shpool,
            tc.tile_pool(name="comb", bufs=3) as combpool,
            tc.tile_pool(name="nodes", bufs=2) as npool,
            tc.tile_pool(name="oh", bufs=2) as ohpool,
            tc.tile_pool(name="outsb", bufs=2) as opool,
            tc.tile_pool(name="ps_agg", bufs=2, space=bass.MemorySpace.PSUM) as ps_agg,
            tc.tile_pool(name="ps_o", bufs=2, space=bass.MemorySpace.PSUM) as ps_o,
            tc.tile_pool(name="ps_g", bufs=1, space=bass.MemorySpace.PSUM) as ps_g,
        ):
            ones4b = cpool.tile([128, 32], BF16)
            wn = cpool.tile([FNODE, FOUT], F32)
            we = cpool.tile([FEP, FOUT], F32)
            waug = cpool.tile([FG + 1, FOUT], F32)
            gat = cpool.tile([FG + 1, NG], F32)
            zeros = cpool.tile([32, 128], F32)
            for sb_t, d in [(ones4b, ones4_d), (wn, wn_d), (we, we_d),
                            (waug, waug_d), (gat, gat_d), (zeros, z_d)]:
                nc.scalar.dma_start(sb_t[:], d[:])

            pg = ps_g.tile([NG, FOUT], F32)
            nc.tensor.matmul(pg[:], gat[:], waug[:], start=True, stop=True)
            g2 = cpool.tile([NG, FOUT], F32)
            nc.vector.tensor_copy(g2[:], pg[:])

            group_tiles = {}
            sh_tiles = {}       # bank -> transposed tile
            passes_by_bank = {}
            for cl in t["chunk_list"]:
                passes_by_bank.setdefault(cl[0], []).append(cl)

            chunk_seq = {"next": 0}
            out_batches = {}  # batch idx -> tile
            ci = {"i": 0}

            def emit_bank(bb):
                nt = t["bank_tiles"][bb]
                p = ps_agg.tile([32, FE * TPB], F32)
                passes = passes_by_bank[bb]
                for idx, (bb_, j, toff, nbj, tcol) in enumerate(passes):
                    gidx, loc = t["chunk_group"][ci["i"]]
                    ci["i"] += 1
                    if gidx not in group_tiles:
                        ts, tl = t["dma_groups"][gidx]
                        hi_t = epool.tile([128, FE * GROUP_TILES], BF16, tag="ehi")
                        lo_t = epool.tile([128, FE * GROUP_TILES], BF16, tag="elo")
                        nc.sync.dma_start(hi_t[:, :FE * tl], ehi_d[:, FE * ts:FE * (ts + tl)])
                        nc.scalar.dma_start(lo_t[:, :FE * tl], elo_d[:, FE * ts:FE * (ts + tl)])
                        group_tiles[gidx] = (hi_t, lo_t)
                    hi_t, lo_t = group_tiles[gidx]
                    dst = p[:, FE * toff:FE * (toff + nbj)]
                    src_hi = hi_t[:, FE * loc:FE * (loc + nbj)]
                    src_lo = lo_t[:, FE * loc:FE * (loc + nbj)]
                    last = idx == len(passes) - 1
                    nc.tensor.matmul(dst, ones4b[:], src_hi, start=(j == 0), stop=False)
                    nc.tensor.matmul(dst, ones4b[:], src_lo, start=False, stop=last)
                # drain (48 -> 64 pitch) + zero the 16-col pads + block transpose
                asb = apool.tile([32, FEP * TPB], F32)
                nc.vector.tensor_copy(
                    asb[:, :FEP * nt].rearrange("p (t f) -> p t f", f=FEP)[:, :, 0:FE],
                    p[:, :FE * nt].rearrange("p (t f) -> p t f", f=FE))
                nc.vector.tensor_copy(
                    asb[:, :FEP * nt].rearrange("p (t f) -> p t f", f=FEP)[:, :, FE:FEP],
                    zeros[:, :16 * nt].rearrange("p (t f) -> p t f", f=16))
                sh = shpool.tile([32, FEP * TPB], F32)
                nc.vector.transpose(sh[:, :FEP * nt], asb[:, :FEP * nt])
                sh_tiles[bb] = sh

            def emit_chunk(c):
                # assemble comb[64, 512] from the two banks' transposed tiles
                comb = combpool.tile([FEP, FINAL_CHUNK], F32)
                for h in range(2):
                    bb = (c * GPC) // TPB + h
                    sh = sh_tiles[bb]
                    # even 32-blocks -> feats 0..31, odd -> feats 32..63
                    src0 = sh[:, 0:FEP * TPB].rearrange("p (t f) -> p t f", f=FEP)[:, :, 0:32]
                    src1 = sh[:, 0:FEP * TPB].rearrange("p (t f) -> p t f", f=FEP)[:, :, 32:64]
                    dst0 = (comb[0:32, 256 * h:256 * (h + 1)]
                            .rearrange("p (t f) -> p t f", f=32))
                    dst1 = (comb[32:64, 256 * h:256 * (h + 1)]
                            .rearrange("p (t f) -> p t f", f=32))
                    nc.vector.tensor_copy(dst0, src0)
                    nc.vector.tensor_copy(dst1, src1)

                bi, off = divmod(c, OUT_BATCH)
                if bi not in out_batches:
                    out_batches[bi] = opool.tile([FOUT, NB], F32, tag="outb", name="outb")
                    nb0 = NB * bi
                    w = min(NB, SN - nb0)
                    nodes_b = npool.tile([FNODE, NB], F32)
                    nc.sync.dma_start(nodes_b[:, :w], nodes_d[:, nb0:nb0 + w])
                    oh_b = ohpool.tile([NG, NB], F32)
                    nc.scalar.dma_start(oh_b[:, :w], oh_d[:, nb0:nb0 + w])
                    out_batches[(bi, "in")] = (nodes_b, oh_b)
                outb = out_batches[bi]
                nodes_b, oh_b = out_batches[(bi, "in")]
                s = slice(FINAL_CHUNK * off, FINAL_CHUNK * (off + 1))

                po = ps_o.tile([FOUT, FINAL_CHUNK], F32)
                nc.tensor.matmul(po[:], wn[:], nodes_b[:, s], start=True, stop=False)
                nc.tensor.matmul(po[:], we[:], comb[:], start=False, stop=False)
                nc.tensor.matmul(po[:], g2[:], oh_b[:, s], start=False, stop=True)
                nc.scalar.activation(outb[:, s], po[:],
                                     mybir.ActivationFunctionType.Relu)
                if off == OUT_BATCH - 1 or c == n_chunks - 1:
                    nb0 = NB * bi
                    w = FINAL_CHUNK * (off + 1)
                    nc.sync.dma_start(out_d[:, nb0:nb0 + w], outb[:, :w])

            for bb in range(n_bf):
                emit_bank(bb)
                while (chunk_seq["next"] + 1) * GPC <= (bb + 1) * TPB:
                    emit_chunk(chunk_seq["next"])
                    chunk_seq["next"] += 1
            while chunk_seq["next"] < n_chunks:
                emit_chunk(chunk_seq["next"])
                chunk_seq["next"] += 1

    nc.compile()
    return nc


def _in_maps(tables, consts, per_core):
    maps = []
    for pc in per_core:
        maps.append({
            "edges_hi": pc["edges_hi"], "edges_lo": pc["edges_lo"],
            "nodes_t": pc["nodes_t"], "onehot_t": pc["onehot_t"],
            "ones4b": consts["ones4b"],
            "W_n": consts["W_n"], "W_e64": consts["W_e64"],
            "W_aug": consts["W_aug"], "gat": consts["gat"],
            "zeros128": consts["zeros128"],
        })
    return maps


def assemble_output(tables, per_core, results):
    N = tables["N"]
    out = np.zeros((N, FOUT), np.float32)
    for pc, r in zip(per_core, results):
        out_t = r["out_t"]
        sn = pc["sn_stream"]
        sel = sn >= 0
        out[sn[sel]] = out_t[:, sel].T
    return out


_CACHE = {}
TRACE = False
LAST_RESULT = None


def kernel(**inputs):
    global LAST_RESULT
    tables, consts, per_core = host_prep(
        inputs["nodes"], inputs["edges"], inputs["globals_"], inputs["W"],
        inputs["b"], inputs["receivers"], inputs["node_graph_ids"])
    key = (tables["SN"], tables["T_ALL"], tables["n_bf"])
    if key not in _CACHE:
        _CACHE[key] = build_program(tables, consts)
    nc = _CACHE[key]
    kw = {}
    if TRACE:
        kw = dict(trace=True, trace_cores=list(range(N_CORES)))
    res = bass_utils.run_bass_kernel_spmd(
        nc, _in_maps(tables, consts, per_core), core_ids=list(range(N_CORES)),
        **kw)
    LAST_RESULT = res
    return assemble_output(tables, per_core, res.results)


def emulate(tables, consts, per_core):
    """Numpy emulation of the device program, from the prepped arrays."""
    t = tables
    outs = []
    g2 = consts["gat"].T @ consts["W_aug"]
    for pc in per_core:
        stream = (pc["edges_hi"].astype(np.float32)
                  + pc["edges_lo"].astype(np.float32)).reshape(128, t["T_ALL"], FE)
        agg = np.zeros((t["SN"], FE), np.float32)
        for (bb, j, toff, nbj, tcol) in t["chunk_list"]:
            g0 = bb * TPB
            data = stream[:, tcol:tcol + nbj, :]
            sums = data.reshape(32, BLOCK, nbj, FE).sum(1)
            for tt in range(nbj):
                gsel = g0 + toff + tt
                agg[32 * gsel:32 * (gsel + 1)] += sums[:, tt, :]
        x = (pc["nodes_t"].T @ consts["W_n"]
             + agg @ consts["W_e64"][:FE]
             + pc["onehot_t"].T @ g2)
        outs.append({"out_t": np.maximum(x, 0).T})
    return assemble_output(t, per_core, outs)
